# revision 23
# baseline (speedup 1.0000x reference)
"""Chamfer distance kernel for Trainium2 (8 NeuronCores, SPMD).

Problem: B=16 batches of two 4096-point 3D clouds; cost =
  sum_b 0.5*(mean_n min_m d2[b,n,m] + mean_m min_n d2[b,n,m]).

Sharding: data-parallel over batch. Each of the 8 cores handles 2 batches;
each batch is ONE pass over the 4096x4096 pair matrix serving BOTH
reduction directions.

Device algorithm (per core, per batch):
  The PE computes H[n,m] = |q_m|^2 - 2 p_n.q_m as a K=4 fp32 matmul with
  lhsT rows (x, y, z, 1) and rhs rows (-2x', -2y', -2z', |q'|^2).
  - direction 1: min_m d2[n,m] = |p_n|^2 + min_m H[n,m]; the row-min is a
    free-axis VectorE reduce, and the |p_n|^2 term is added on the host.
  - direction 2: min_n d2[n,m] = min_n (H[n,m] + |p_n|^2); a
    scalar_tensor_tensor op folds the per-partition |p_n|^2 bias into a
    running column-min accumulator (ping-pong colA/colB), which is
    finally PE-transposed and min-reduced across partitions.
  Coordinates arrive as int16 fixed-point (scale 2^-11; quantization
  perturbs d2 by ~1e-4, far inside the 2e-2 gate) and are dequantized
  on-device; |q|^2 is built on-device via a ones-matmul partition sum.

  The compute sits in hardware For_i loops (~55 static instructions):
  small NEFF, small per-call load/ship cost, one small input tensor.
"""

import sys

sys.path.insert(0, "/opt/trn_rl_repo")

from contextlib import ExitStack

import numpy as np

import concourse.bass as bass
import concourse.tile as tile
from concourse import bacc, mybir
from concourse.bass_utils import run_bass_kernel_spmd
from concourse.masks import make_identity

B, N, D = 16, 4096, 3
NCORES = 8
BPC = B // NCORES      # batches per core
NSEC = 2 * BPC         # output sections per core: (batch, direction)
K = 4                  # contraction rows: x, y, z, const
NALL = NSEC * N        # 16384 points per core (4 clouds)
CPAD = NALL + 129      # + (-2)-column + 128 ones
QSCALE = 2048.0
BIGF = 3.0e38
AVG_SCALE = 0.5
F32 = mybir.dt.float32
MIN = mybir.AluOpType.min
ADD = mybir.AluOpType.add
X = mybir.AxisListType.X

_NC = {}


def _build():
    nc = bacc.Bacc("TRN2", target_bir_lowering=False, debug=False)
    # src rows (x, y, z) as int16 fixed-point (scale 2^-11), free axis =
    # (cloud g, point n); cloud g = 2*b_local + {0: points1, 1: points2}.
    # Constant pad columns: col NALL = -2*2048, cols NALL+1.. = 2048.
    src = nc.dram_tensor("src", [D, CPAD], mybir.dt.int16,
                         kind="ExternalInput").ap()
    # out[(2*bl+d)*128 + p]: d=0 row partial sums, d=1 col partial sums
    out = nc.dram_tensor("out", [NSEC * 128], F32, kind="ExternalOutput").ap()
    # HBM scratch for |q|^2 (dynamic-dst DMA must target DRAM, not SBUF)
    sqd = nc.dram_tensor("sqd", [NALL], F32, kind="Internal").ap()

    with tile.TileContext(nc) as tc, ExitStack() as ctx:
        sb = ctx.enter_context(tc.tile_pool(name="sb", bufs=1))
        ps = ctx.enter_context(tc.tile_pool(name="ps", bufs=1, space="PSUM"))
        SI = sb.tile([D, CPAD], mybir.dt.int16, tag="SI")
        S = sb.tile([D, CPAD], F32, tag="S")
        R = sb.tile([K, BPC * N], F32, tag="R")
        colA = sb.tile([128, N], F32, tag="colA")
        colB = sb.tile([128, N], F32, tag="colB")
        SQL = sb.tile([128, BPC * 32], F32, tag="SQL")
        ident = sb.tile([128, 128], F32, tag="ident")
        s2c = [sb.tile([D, 512], F32, name=f"s2c{u}", tag=f"s2c{u}")
               for u in range(2)]
        sqc = [sb.tile([1, 512], F32, name=f"sqc{u}", tag=f"sqc{u}")
               for u in range(2)]
        scal = sb.tile([D, 1], F32, tag="scal")
        ones3 = sb.tile([D, 1], F32, tag="ones3")
        nc.sync.dma_start(SI[:], src)
        nc.scalar.mul(S[:], SI[:], 1.0 / QSCALE)
        nc.sync.dma_start(scal[:], S[0:D, NALL:NALL + 1])
        nc.sync.dma_start(ones3[:], S[0:D, NALL + 1:NALL + 2])
        pp = [ps.tile([128, 2048], F32, name=f"pp{h}", tag=f"pp{h}")
              for h in range(2)]

        # |q|^2 per point: ones-matmul partition sum of squared coords,
        # staged through HBM (dynamic-dst DMA only works to DRAM).
        # Two ping-pong lanes per iteration so the PE->ScalarE->DMA chain
        # of lane A overlaps lane B instead of serializing 32 deep.
        with tc.For_i(0, NALL, 1024) as t:
            for u in range(2):
                nc.vector.tensor_tensor(
                    s2c[u][:], S[0:D, bass.ds(t + u * 512, 512)],
                    S[0:D, bass.ds(t + u * 512, 512)],
                    op=mybir.AluOpType.mult)
                nc.tensor.matmul(pp[u][0:1, 0:512], ones3[:], s2c[u][:],
                                 start=True, stop=True)
                nc.scalar.copy(sqc[u][:], pp[u][0:1, 0:512])
                nc.sync.dma_start(sqd[bass.ds(t + u * 512, 512)], sqc[u][:])
        # Batch bl rhs = cloud 2bl+1: rows 0:3 = -2 coords, row 3 = |q|^2;
        # SQL col (bl*32+t)[p] = |p|^2 of L-cloud-(2bl) point t*128+p.
        for bl in range(BPC):
            a, b = bl * N, (2 * bl + 1) * N
            g0 = 2 * bl * N
            nc.vector.tensor_scalar(R[0:D, a:a + N], S[0:D, b:b + N],
                                    scal[:, 0:1], None,
                                    mybir.AluOpType.mult)
            nc.sync.dma_start(R[3:4, a:a + N], sqd[b:b + N])
            nc.sync.dma_start(
                SQL[:, bl * 32:(bl + 1) * 32],
                sqd[g0:g0 + N].rearrange("(t p) -> p t", p=128))
        make_identity(nc, ident[:])

        wb = [sb.tile([K, 256], F32, name=f"wb{p}", tag=f"wb{p}")
              for p in range(2)]
        for p in range(2):
            for u in range(2):
                nc.sync.dma_start(wb[p][3:4, u * 128:(u + 1) * 128],
                                  S[0:1, NALL + 1:NALL + 129])
        rexA = sb.tile([128, 32], F32, tag="rexA")
        rexB = sb.tile([128, 32], F32, tag="rexB")
        rext = sb.tile([128, 32], F32, tag="rext")
        rsum = sb.tile([128, 1], F32, tag="rsum")
        CM = sb.tile([128, 32], F32, tag="CM")
        csum = sb.tile([128, 1], F32, tag="csum")

        for bl in range(BPC):
            g0 = 2 * bl * N
            nc.vector.memset(colA[:], BIGF)
            with tc.For_i(0, 8, 1) as kk:
                for p in range(2):
                    # lhsT pair: 2x128 L-cloud points per wide copy
                    nc.scalar.copy(
                        wb[p][0:D, :],
                        S[0:D, bass.ds(g0 + kk * 512 + p * 256, 256)])
                    for u in range(2):
                        ug = p * 2 + u  # tile index within iteration
                        cin, cout = (colA, colB) if ug % 2 == 0 \
                            else (colB, colA)
                        sqlcol = SQL[:, bass.ds(bl * 32 + kk * 4 + ug, 1)]
                        for h in range(2):
                            for mc in range(4):
                                nc.tensor.matmul(
                                    pp[h][:, mc * 512:(mc + 1) * 512],
                                    wb[p][:, u * 128:(u + 1) * 128],
                                    R[:, bass.ds(bl * N + h * 2048
                                                 + mc * 512, 512)],
                                    start=True, stop=True)
                            rex = rexA if h == 0 else rexB
                            nc.vector.tensor_reduce(
                                rex[:, bass.ds(kk * 4 + ug, 1)], pp[h][:],
                                op=MIN, axis=X)
                            hc = slice(h * 2048, (h + 1) * 2048)
                            nc.vector.scalar_tensor_tensor(
                                cout[:, hc], pp[h][:], sqlcol, cin[:, hc],
                                ADD, MIN)
            # direction-1 output: sum over row tiles of row minima
            nc.vector.tensor_tensor(rext[:], rexA[:], rexB[:], op=MIN)
            nc.vector.tensor_reduce(rsum[:], rext[:], op=ADD, axis=X)
            nc.sync.dma_start(out[(2 * bl) * 128:(2 * bl) * 128 + 128],
                              rsum[:])
            # direction-2 output: transpose colA, min across partitions
            # (python-unrolled: ldweights needs static offsets)
            for t in range(32):
                nc.tensor.transpose(pp[t % 2][:, 0:128],
                                    colA[:, t * 128:(t + 1) * 128],
                                    ident[:])
                nc.vector.tensor_reduce(CM[:, t:t + 1],
                                        pp[t % 2][:, 0:128], op=MIN, axis=X)
            nc.vector.tensor_reduce(csum[:], CM[:], op=ADD, axis=X)
            nc.sync.dma_start(
                out[(2 * bl + 1) * 128:(2 * bl + 1) * 128 + 128], csum[:])

    nc.compile()
    return nc


def get_nc(mode=None):
    if "nc" not in _NC:
        _NC["nc"] = _build()
    return _NC["nc"]


def _quant(points):
    """[B, N, 3] f32 -> int16 fixed-point (scale 2^-11)."""
    p = np.asarray(points, dtype=np.float32)
    return np.clip(np.rint(p * QSCALE), -32768, 32767).astype(np.int16)


def _prep_inputs(points1, points2, mode=None):
    """points1/2 [B, N, 3] f32 -> per-core {"src": [3, CPAD] i16} maps."""
    q1, q2 = _quant(points1), _quant(points2)
    maps = []
    for c in range(NCORES):
        src = np.full((D, CPAD), int(QSCALE), dtype=np.int16)
        src[:, NALL] = -2 * int(QSCALE)
        for bl in range(BPC):
            gb = c * BPC + bl
            for ci, pts in ((0, q1[gb]), (1, q2[gb])):
                g = 2 * bl + ci
                src[:, g * N:(g + 1) * N] = pts.T
        maps.append({"src": src})
    return maps


def _sumsq(points):
    # |p|^2 sums of the QUANTIZED clouds, matching the device's values.
    p = _quant(points).astype(np.float64) / QSCALE
    return np.sum(p * p, axis=(1, 2))  # [B]


def _assemble(results, points1, points2):
    ss1 = _sumsq(points1)
    total = 0.0
    for c in range(NCORES):
        r = results[c]["out"].astype(np.float64).reshape(NSEC, 128)
        for bl in range(BPC):
            gb = c * BPC + bl
            m1 = (r[2 * bl].sum() + ss1[gb]) / N    # mean_n min_m d2
            m2 = r[2 * bl + 1].sum() / N            # mean_m min_n d2
            total += AVG_SCALE * (m1 + m2)
    return np.asarray(total, dtype=np.float32)


def run(points1, points2, trace=False, tmpdir=None, mode=None):
    nc = get_nc()
    in_maps = _prep_inputs(points1, points2)
    res = run_bass_kernel_spmd(nc, in_maps, list(range(NCORES)),
                               trace=trace, tmpdir=tmpdir)
    return _assemble(res.results, points1, points2), res


def kernel(points1, points2):
    out, _ = run(points1, points2)
    return out


# revision 24
# speedup vs baseline: 1.8080x; 1.8080x over previous
"""Chamfer distance kernel for Trainium2 (8 NeuronCores, SPMD).

Problem: B=16 batches of two 4096-point 3D clouds; cost =
  sum_b 0.5*(mean_n min_m d2[b,n,m] + mean_m min_n d2[b,n,m]).

Sharding: data-parallel over batch. Each of the 8 cores handles 2 batches;
each batch is ONE pass over the 4096x4096 pair matrix serving BOTH
reduction directions.

Device algorithm (per core, per batch):
  The PE computes H[n,m] = |q_m|^2 - 2 p_n.q_m as a K=4 fp32 matmul with
  lhsT rows (x, y, z, 1) and rhs rows (-2x', -2y', -2z', |q'|^2).
  - direction 1: min_m d2[n,m] = |p_n|^2 + min_m H[n,m]; the row-min is a
    free-axis VectorE reduce, and the |p_n|^2 term is added on the host.
  - direction 2: min_n d2[n,m] = min_n (H[n,m] + |p_n|^2); a
    scalar_tensor_tensor op folds the per-partition |p_n|^2 bias into a
    running column-min accumulator (ping-pong colA/colB), which is
    finally PE-transposed and min-reduced across partitions.
  Coordinates arrive as int16 fixed-point (scale 2^-11; quantization
  perturbs d2 by ~1e-4, far inside the 2e-2 gate) and are dequantized
  on-device; |q|^2 is built on-device via a ones-matmul partition sum.

  The compute sits in hardware For_i loops (~55 static instructions):
  small NEFF, small per-call load/ship cost, one small input tensor.
"""

import sys

sys.path.insert(0, "/opt/trn_rl_repo")

from contextlib import ExitStack

import numpy as np

import concourse.bass as bass
import concourse.tile as tile
from concourse import bacc, mybir
from concourse.bass_utils import run_bass_kernel_spmd
from concourse.masks import make_identity

B, N, D = 16, 4096, 3
NCORES = 8
BPC = B // NCORES      # batches per core
NSEC = 2 * BPC         # output sections per core: (batch, direction)
K = 4                  # contraction rows: x, y, z, const
NALL = NSEC * N        # 16384 points per core (4 clouds)
CPAD = NALL + 129      # + (-2)-column + 128 ones
QSCALE = 2048.0
BIGF = 3.0e38
AVG_SCALE = 0.5
F32 = mybir.dt.float32
MIN = mybir.AluOpType.min
ADD = mybir.AluOpType.add
X = mybir.AxisListType.X

_NC = {}


def _build():
    nc = bacc.Bacc("TRN2", target_bir_lowering=False, debug=False)
    # src rows (x, y, z) as int16 fixed-point (scale 2^-11), free axis =
    # (cloud g, point n); cloud g = 2*b_local + {0: points1, 1: points2}.
    # Constant pad columns: col NALL = -2*2048, cols NALL+1.. = 2048.
    src = nc.dram_tensor("src", [D, CPAD], mybir.dt.int16,
                         kind="ExternalInput").ap()
    # out[(2*bl+d)*128 + p]: d=0 row partial sums, d=1 col partial sums
    out = nc.dram_tensor("out", [NSEC * 128], F32, kind="ExternalOutput").ap()
    # HBM scratch for |q|^2 (dynamic-dst DMA must target DRAM, not SBUF)
    sqd = nc.dram_tensor("sqd", [NALL], F32, kind="Internal").ap()

    with tile.TileContext(nc) as tc, ExitStack() as ctx:
        sb = ctx.enter_context(tc.tile_pool(name="sb", bufs=1))
        ps = ctx.enter_context(tc.tile_pool(name="ps", bufs=1, space="PSUM"))
        SI = sb.tile([D, CPAD], mybir.dt.int16, tag="SI")
        S = sb.tile([D, CPAD], F32, tag="S")
        R = sb.tile([K, BPC * N], F32, tag="R")
        colA = sb.tile([128, N], F32, tag="colA")
        colB = sb.tile([128, N], F32, tag="colB")
        SQL = sb.tile([128, BPC * 32], F32, tag="SQL")
        ident = sb.tile([128, 128], F32, tag="ident")
        s2c = [sb.tile([D, 512], F32, name=f"s2c{u}", tag=f"s2c{u}")
               for u in range(2)]
        sqc = [sb.tile([1, 512], F32, name=f"sqc{u}", tag=f"sqc{u}")
               for u in range(2)]
        scal = sb.tile([D, 1], F32, tag="scal")
        ones3 = sb.tile([D, 1], F32, tag="ones3")
        nc.sync.dma_start(SI[:], src)
        nc.scalar.mul(S[:], SI[:], 1.0 / QSCALE)
        nc.sync.dma_start(scal[:], S[0:D, NALL:NALL + 1])
        nc.sync.dma_start(ones3[:], S[0:D, NALL + 1:NALL + 2])
        pp = [ps.tile([128, 2048], F32, name=f"pp{h}", tag=f"pp{h}")
              for h in range(2)]

        # |q|^2 per point: ones-matmul partition sum of squared coords,
        # staged through HBM (dynamic-dst DMA only works to DRAM).
        # Two ping-pong lanes per iteration so the PE->ScalarE->DMA chain
        # of lane A overlaps lane B instead of serializing 32 deep.
        with tc.For_i(0, NALL, 1024) as t:
            for u in range(2):
                nc.vector.tensor_tensor(
                    s2c[u][:], S[0:D, bass.ds(t + u * 512, 512)],
                    S[0:D, bass.ds(t + u * 512, 512)],
                    op=mybir.AluOpType.mult)
                nc.tensor.matmul(pp[u][0:1, 0:512], ones3[:], s2c[u][:],
                                 start=True, stop=True)
                nc.scalar.copy(sqc[u][:], pp[u][0:1, 0:512])
                nc.sync.dma_start(sqd[bass.ds(t + u * 512, 512)], sqc[u][:])
        # Batch bl rhs = cloud 2bl+1: rows 0:3 = -2 coords, row 3 = |q|^2;
        # SQL col (bl*32+t)[p] = |p|^2 of L-cloud-(2bl) point t*128+p.
        for bl in range(BPC):
            a, b = bl * N, (2 * bl + 1) * N
            g0 = 2 * bl * N
            nc.vector.tensor_scalar(R[0:D, a:a + N], S[0:D, b:b + N],
                                    scal[:, 0:1], None,
                                    mybir.AluOpType.mult)
            nc.sync.dma_start(R[3:4, a:a + N], sqd[b:b + N])
            nc.sync.dma_start(
                SQL[:, bl * 32:(bl + 1) * 32],
                sqd[g0:g0 + N].rearrange("(t p) -> p t", p=128))
        make_identity(nc, ident[:])

        wb = [sb.tile([K, 256], F32, name=f"wb{p}", tag=f"wb{p}")
              for p in range(2)]
        for p in range(2):
            for u in range(2):
                nc.sync.dma_start(wb[p][3:4, u * 128:(u + 1) * 128],
                                  S[0:1, NALL + 1:NALL + 129])
        rexA = sb.tile([128, 32], F32, tag="rexA")
        rexB = sb.tile([128, 32], F32, tag="rexB")
        rext = sb.tile([128, 32], F32, tag="rext")
        rsum = sb.tile([128, 1], F32, tag="rsum")
        CM = sb.tile([128, 32], F32, tag="CM")
        csum = sb.tile([128, 1], F32, tag="csum")

        for bl in range(BPC):
            g0 = 2 * bl * N
            nc.vector.memset(colA[:], BIGF)
            with tc.For_i(0, 8, 1) as kk:
                for p in range(2):
                    # lhsT pair: 2x128 L-cloud points per wide copy
                    nc.scalar.copy(
                        wb[p][0:D, :],
                        S[0:D, bass.ds(g0 + kk * 512 + p * 256, 256)])
                    for u in range(2):
                        ug = p * 2 + u  # tile index within iteration
                        cin, cout = (colA, colB) if ug % 2 == 0 \
                            else (colB, colA)
                        sqlcol = SQL[:, bass.ds(bl * 32 + kk * 4 + ug, 1)]
                        for h in range(2):
                            for mc in range(4):
                                nc.tensor.matmul(
                                    pp[h][:, mc * 512:(mc + 1) * 512],
                                    wb[p][:, u * 128:(u + 1) * 128],
                                    R[:, bass.ds(bl * N + h * 2048
                                                 + mc * 512, 512)],
                                    start=True, stop=True)
                            rex = rexA if h == 0 else rexB
                            nc.vector.tensor_reduce(
                                rex[:, bass.ds(kk * 4 + ug, 1)], pp[h][:],
                                op=MIN, axis=X)
                            hc = slice(h * 2048, (h + 1) * 2048)
                            nc.vector.scalar_tensor_tensor(
                                cout[:, hc], pp[h][:], sqlcol, cin[:, hc],
                                ADD, MIN)
            # direction-1 output: sum over row tiles of row minima
            nc.vector.tensor_tensor(rext[:], rexA[:], rexB[:], op=MIN)
            nc.vector.tensor_reduce(rsum[:], rext[:], op=ADD, axis=X)
            nc.sync.dma_start(out[(2 * bl) * 128:(2 * bl) * 128 + 128],
                              rsum[:])
            # direction-2 output: transpose colA, min across partitions
            # (python-unrolled: ldweights needs static offsets). Four
            # transposed blocks share one PSUM tile; a single 3D-view
            # reduce collapses each block's partition axis.
            for g in range(8):
                pg = pp[g % 2]
                for j in range(4):
                    t = g * 4 + j
                    nc.tensor.transpose(pg[:, j * 128:(j + 1) * 128],
                                        colA[:, t * 128:(t + 1) * 128],
                                        ident[:])
                view = pg[:, 0:512].rearrange("p (a b) -> p a b", b=128)
                nc.vector.tensor_reduce(CM[:, g * 4:(g + 1) * 4], view,
                                        op=MIN, axis=X)
            nc.vector.tensor_reduce(csum[:], CM[:], op=ADD, axis=X)
            nc.sync.dma_start(
                out[(2 * bl + 1) * 128:(2 * bl + 1) * 128 + 128], csum[:])

    nc.compile()
    return nc


def get_nc(mode=None):
    if "nc" not in _NC:
        _NC["nc"] = _build()
    return _NC["nc"]


def _quant(points):
    """[B, N, 3] f32 -> int16 fixed-point (scale 2^-11)."""
    p = np.asarray(points, dtype=np.float32)
    return np.clip(np.rint(p * QSCALE), -32768, 32767).astype(np.int16)


def _prep_inputs(points1, points2, mode=None):
    """points1/2 [B, N, 3] f32 -> per-core {"src": [3, CPAD] i16} maps."""
    q1, q2 = _quant(points1), _quant(points2)
    maps = []
    for c in range(NCORES):
        src = np.full((D, CPAD), int(QSCALE), dtype=np.int16)
        src[:, NALL] = -2 * int(QSCALE)
        for bl in range(BPC):
            gb = c * BPC + bl
            for ci, pts in ((0, q1[gb]), (1, q2[gb])):
                g = 2 * bl + ci
                src[:, g * N:(g + 1) * N] = pts.T
        maps.append({"src": src})
    return maps


def _sumsq(points):
    # |p|^2 sums of the QUANTIZED clouds, matching the device's values.
    p = _quant(points).astype(np.float64) / QSCALE
    return np.sum(p * p, axis=(1, 2))  # [B]


def _assemble(results, points1, points2):
    ss1 = _sumsq(points1)
    total = 0.0
    for c in range(NCORES):
        r = results[c]["out"].astype(np.float64).reshape(NSEC, 128)
        for bl in range(BPC):
            gb = c * BPC + bl
            m1 = (r[2 * bl].sum() + ss1[gb]) / N    # mean_n min_m d2
            m2 = r[2 * bl + 1].sum() / N            # mean_m min_n d2
            total += AVG_SCALE * (m1 + m2)
    return np.asarray(total, dtype=np.float32)


def run(points1, points2, trace=False, tmpdir=None, mode=None):
    nc = get_nc()
    in_maps = _prep_inputs(points1, points2)
    res = run_bass_kernel_spmd(nc, in_maps, list(range(NCORES)),
                               trace=trace, tmpdir=tmpdir)
    return _assemble(res.results, points1, points2), res


def kernel(points1, points2):
    out, _ = run(points1, points2)
    return out
